# revision 40
# baseline (speedup 1.0000x reference)
"""Single-head causal attention (B=8, T=2048, C=1024, H=128) on 8 TRN2 NeuronCores.

Sharding: data-parallel over batch B — core b computes attention for x[b].
Host-side prep per core: x[b] is transposed to xT [C, T] (contraction dim C on
SBUF partitions) and the softmax scale C**-0.5 is folded into Wq. The kernel
returns the UNNORMALIZED attention output pavT [H, T] (bf16) plus the softmax
denominators sums [1, T] (f32); the host divides and untransposes.

Device kernel per core (ST-direct, projections interleaved with attention):
  quarter 0 projections up front (V,Q,K c-major over arriving x chunks).
  per q-block g, per causal s-tile j (suffix-trimmed to the valid q-range):
      ST_j = KT_j.T @ QT_g[suffix]    [s=128, N<=512] PSUM  (PE)
      diag boundary tile gets a [128,128] triangular mask add (DVE)
      expST = exp(ST)                  -> SBUF bf16          (ACT)
      pav_g += V_j.T @ expST           [H, 512] PSUM         (PE)
      acc_{e,o} += expST               bf16 partial sums     (DVE / GpSimd)
      interleaved projection matmuls fill the exp latency    (PE)
  softmax denominators: acc_e+acc_o -> ones-column matmul [1,512] (deferred
  into the next block), row -> SBUF -> DMA (scalar). pav -> SBUF bf16 (ACT),
  DMA out (sync). Quarter q's Q,V are projected during attn(q-1); K during
  attn(q) s-tiles j < 4q.
"""

from contextlib import ExitStack

import numpy as np
import ml_dtypes

B, T, C, H = 8, 2048, 1024, 128
P = 128
NT = T // P  # 16 s-tiles
NCC = C // P  # 8 contraction chunks
QB = 512  # q-block width
NQB = T // QB  # 4 q-blocks / projection quarters
N_CORES = 8
SCALE = float(C) ** -0.5

_CACHE = {}


def _build():
    import concourse.bass as bass
    import concourse.mybir as mybir
    import concourse.tile as tile
    from concourse import bacc

    dt = mybir.dt
    dt_in = dt.bfloat16
    dt_av = dt.bfloat16
    f32 = dt.float32

    nc = bacc.Bacc("TRN2", target_bir_lowering=False, debug=False)
    xT = nc.dram_tensor("xT", [C, T], dt_in, kind="ExternalInput").ap()
    wq = nc.dram_tensor("wq", [P, NCC * H], dt_in, kind="ExternalInput").ap()
    wk = nc.dram_tensor("wk", [P, NCC * H], dt_in, kind="ExternalInput").ap()
    wv = nc.dram_tensor("wv", [P, NCC * H], dt_in, kind="ExternalInput").ap()
    pavT = nc.dram_tensor("pavT", [H, T], dt_av, kind="ExternalOutput").ap()
    sums = nc.dram_tensor("sums", [1, T], f32, kind="ExternalOutput").ap()

    with tile.TileContext(nc) as tc, ExitStack() as ctx:
        # --- weights + first x quarter: priority DMAs over three queues ---
        wpool = ctx.enter_context(tc.tile_pool(name="wpool", bufs=1))
        w_sb = {
            name: wpool.tile([P, NCC * H], dt_in, name=f"{name}_sb")
            for name in ("wv", "wq", "wk")
        }
        xpool = ctx.enter_context(tc.tile_pool(name="xpool", bufs=1))
        xt_sb = xpool.tile([P, NCC * T], dt_in)

        def x_chunk(c, c0, c1, eng):
            eng.dma_start(
                xt_sb[:, c * T + c0 : c * T + c1],
                xT[c * P : (c + 1) * P, c0:c1],
            )

        nc.sync.dma_start(w_sb["wv"], wv)
        nc.scalar.dma_start(w_sb["wq"], wq)
        n0_eng = [nc.sync, nc.gpsimd, nc.scalar, nc.sync,
                  nc.gpsimd, nc.scalar, nc.sync, nc.gpsimd]
        for c in range(NCC):
            x_chunk(c, 0, 512, n0_eng[c])
        nc.scalar.dma_start(w_sb["wk"], wk)
        # n=1 and n=2..3 are single batched strided DMAs, deferred so they
        # don't steal HBM bandwidth from the n=0 chunks gating quarter 0
        xt_v = xt_sb.rearrange("p (c t) -> p c t", c=NCC)
        xT_v = xT.rearrange("(c p) t -> p c t", c=NCC)

        consts = ctx.enter_context(tc.tile_pool(name="consts", bufs=1))
        # triangular boundary mask: tri[s, q] = -30000 where q < s else 0
        tri = consts.tile([P, P], f32)
        nc.gpsimd.memset(tri, 0.0)
        nc.gpsimd.affine_select(
            out=tri,
            in_=tri,
            compare_op=mybir.AluOpType.is_ge,
            fill=-30000.0,
            base=0,
            pattern=[[1, P]],
            channel_multiplier=-1,
        )
        ones_col = consts.tile([P, 1], dt_av)
        nc.vector.memset(ones_col, 1.0)

        qkv = ctx.enter_context(tc.tile_pool(name="qkv", bufs=1))
        qt_sb = qkv.tile([P, T], dt_in)
        kt_sb = qkv.tile([P, T], dt_in)
        vt_sb = qkv.tile([P, T], dt_av)
        vpool = ctx.enter_context(tc.tile_pool(name="vpool", bufs=1))
        v_sb = vpool.tile([P, NT * H], dt_av)

        # PSUM banks: scores x3, proj x2, pav x2, sums-final x1 -> 8
        # (proj needs 2: with 1 bank the next projection's matmuls stall on
        # the previous quarter-copy draining the bank behind queued exps)
        ps_pool = ctx.enter_context(tc.tile_pool(name="ps_pool", bufs=3, space="PSUM"))
        ps_proj = ctx.enter_context(tc.tile_pool(name="ps_proj", bufs=2, space="PSUM"))
        ps_av = ctx.enter_context(tc.tile_pool(name="ps_av", bufs=2, space="PSUM"))
        ps_sums = ctx.enter_context(
            tc.tile_pool(name="ps_sums", bufs=1, space="PSUM")
        )

        expst_pool = ctx.enter_context(tc.tile_pool(name="expst_pool", bufs=8))
        outp = ctx.enter_context(tc.tile_pool(name="outp", bufs=2))
        accp = ctx.enter_context(tc.tile_pool(name="accp", bufs=4))
        sums_sb_pool = ctx.enter_context(tc.tile_pool(name="sums_sb", bufs=1))
        sums_sb = sums_sb_pool.tile([1, T], f32)

        def proj_ops(pname, dst, n, copy_eng, pool, do_transpose):
            """Closures: 8 proj matmuls + copy (+ quarter transpose)."""
            wt = w_sb[pname]
            state = {}

            def mk_mm(c):
                def op():
                    if c == 0:
                        state["ps"] = pool.tile(
                            [P, QB], f32, name=f"ps_{pname}{n}", tag=pool.name
                        )
                    nc.tensor.matmul(
                        state["ps"],
                        wt[:, c * H : (c + 1) * H],
                        xt_sb[:, c * T + n * QB : c * T + (n + 1) * QB],
                        start=(c == 0),
                        stop=(c == NCC - 1),
                    )

                return op

            def cp():
                fn = (
                    copy_eng.tensor_copy if copy_eng is nc.vector else copy_eng.copy
                )
                fn(dst[:, n * QB : (n + 1) * QB], state["ps"])

            ops = [mk_mm(c) for c in range(NCC)] + [cp]
            if do_transpose:

                def tr():
                    nc.sync.dma_start(
                        v_sb[:, 4 * n * H : 4 * (n + 1) * H].rearrange(
                            "p (t h) -> p t h", t=4
                        ),
                        vt_sb[:, n * QB : (n + 1) * QB],
                        transpose=True,
                    )

                ops.append(tr)
            return ops

        # --- quarter 0 up front: V,Q,K c-major over arriving x chunks ---
        v0 = proj_ops("wv", vt_sb, 0, nc.scalar, ps_proj, True)
        q0 = proj_ops("wq", qt_sb, 0, nc.vector, ps_pool, False)
        k0 = proj_ops("wk", kt_sb, 0, nc.vector, ps_pool, False)
        for c in range(NCC):
            v0[c]()
            q0[c]()
            k0[c]()
            if c == 3:  # n=1 lower half: c-chunks 0-3
                nc.sync.dma_start(
                    xt_v[:, 0:4, 512:1024], xT_v[:, 0:4, 512:1024]
                )
            if c == 5:  # n=1 upper half: c-chunks 4-7
                nc.sync.dma_start(
                    xt_v[:, 4:8, 512:1024], xT_v[:, 4:8, 512:1024]
                )
        q0[8]()  # Q copy (DVE)
        k0[8]()  # K copy (DVE)
        v0[8]()  # V copy (ACT)
        v0[9]()  # transpose (sync)

        # --- attention blocks ---
        deferred = []  # sums finalization + outputs of the previous block
        for g in range(NQB):
            qs0 = g * QB
            njt = 4 * g + 4
            pav = ps_av.tile([P, QB], f32, name=f"pav{g}", tag="ps_av")
            acc_e = accp.tile([P, QB], dt_av, name=f"acce{g}", tag="acc")
            acc_o = accp.tile([P, QB], dt_av, name=f"acco{g}", tag="acc")

            # one deadline-ordered op queue, spread evenly over the block:
            # K(g) first (needed at s-tile 4g), then Q(g+1) (next block's
            # start), then V(g+1) (consumed only at the END of block g+1)
            ops_q = []
            if g >= 1:
                ops_q += proj_ops("wk", kt_sb, g, nc.vector, ps_proj, False)
            if g + 1 < NQB:
                ops_q += proj_ops("wq", qt_sb, g + 1, nc.vector, ps_proj, False)
                ops_q += proj_ops("wv", vt_sb, g + 1, nc.scalar, ps_proj, True)
            # last block: diagonal tiles mid-block so the block (and kernel)
            # ends with dense full tiles instead of mask->exp chain drains
            if g == NQB - 1:
                order = list(range(8)) + [12, 13, 14, 15] + [8, 9, 10, 11]
                win = 8  # K(g) copy must land before s-tile 12 at idx 8
            else:
                order = list(range(njt))
                win = njt - 1
            for idx in range(njt):
                j = order[idx]
                d = j - 4 * g
                qlo = max(0, P * d)
                ps = ps_pool.tile([P, QB], f32, name=f"ps_{g}_{j}", tag="ps_pool")
                nc.tensor.matmul(
                    ps[:, qlo:QB],
                    kt_sb[:, j * P : (j + 1) * P],
                    qt_sb[:, qs0 + qlo : qs0 + QB],
                    start=True,
                    stop=True,
                )
                if d >= 0:
                    nc.vector.tensor_add(
                        ps[:, qlo : qlo + P], ps[:, qlo : qlo + P], tri
                    )
                expst = expst_pool.tile(
                    [P, QB], dt_av, name=f"expst{g}_{j}", tag="expst"
                )
                nc.scalar.activation(
                    expst[:, qlo:QB],
                    ps[:, qlo:QB],
                    mybir.ActivationFunctionType.Exp,
                )
                # interleave projections into the exp latency window
                if g == 0 and idx == 1:  # n=2..3 batched x load
                    nc.gpsimd.dma_start(
                        xt_v[:, :, 1024:2048], xT_v[:, :, 1024:2048]
                    )
                if idx == 2 and deferred:
                    for op in deferred:
                        op()
                    deferred = []
                if ops_q and idx < win:
                    take = -(-len(ops_q) // (win - idx))
                    for op in ops_q[:take]:
                        op()
                    ops_q = ops_q[take:]
                nc.tensor.matmul(
                    pav[:, qlo:QB],
                    v_sb[:, j * H : (j + 1) * H],
                    expst[:, qlo:QB],
                    start=(idx == 0),
                    stop=(idx == njt - 1),
                    skip_group_check=True,
                )
                # running exp-sums on DVE (even idx) / GpSimd (odd idx); the
                # final tile goes to DVE so the tail chain avoids a GpSimd hop
                on_dve = idx % 2 == 0 or (g == NQB - 1 and idx == njt - 1)
                eng = nc.vector if on_dve else nc.gpsimd
                acc = acc_e if on_dve else acc_o
                if idx < 2:
                    if qlo > 0:
                        eng.memset(acc[:, 0:qlo], 0.0)
                    eng.tensor_copy(acc[:, qlo:QB], expst[:, qlo:QB])
                else:
                    eng.tensor_add(
                        acc[:, qlo:QB], acc[:, qlo:QB], expst[:, qlo:QB]
                    )
            for op in ops_q:  # leftovers (shouldn't happen)
                op()

            def mk_finalize(g=g, qs0=qs0, acc_e=acc_e, acc_o=acc_o, pav=pav):
                def fin():
                    nc.vector.tensor_add(acc_e, acc_e, acc_o)
                    pss = ps_sums.tile(
                        [1, QB], f32, name=f"pss{g}", tag="ps_sums"
                    )
                    nc.tensor.matmul(pss, ones_col, acc_e, start=True, stop=True)
                    nc.vector.tensor_copy(sums_sb[:, qs0 : qs0 + QB], pss)
                    nc.scalar.dma_start(
                        sums[:, qs0 : qs0 + QB], sums_sb[:, qs0 : qs0 + QB]
                    )
                    o = outp.tile([P, QB], dt_av, name=f"o{g}", tag="o")
                    nc.scalar.copy(o, pav)
                    nc.sync.dma_start(pavT[:, qs0 : qs0 + QB], o)

                return fin

            deferred = [mk_finalize()]
        for op in deferred:
            op()

    nc.compile()
    return nc


def _get_bass():
    if "nc" not in _CACHE:
        _CACHE["nc"] = _build()
    return _CACHE["nc"]


LAST_RESULT = None  # BassKernelResults of the most recent kernel() call


def _make_in_maps(x, Wq, Wk, Wv):
    np_dt = ml_dtypes.bfloat16

    def _wlayout(w):  # [C, H] -> [P, NCC*H]: sbuf layout, contiguous DMA
        w = np.asarray(w, np.float32).reshape(NCC, P, H).transpose(1, 0, 2)
        return np.ascontiguousarray(w.reshape(P, NCC * H)).astype(np_dt)

    wq_s = _wlayout(np.asarray(Wq, np.float32) * SCALE)
    wk_s = _wlayout(Wk)
    wv_s = _wlayout(Wv)
    x = np.asarray(x, np.float32)

    in_maps = []
    for b in range(N_CORES):
        in_maps.append(
            {
                "xT": np.ascontiguousarray(x[b].T).astype(np_dt),
                "wq": wq_s,
                "wk": wk_s,
                "wv": wv_s,
            }
        )
    return in_maps


def _finalize(pavT_arr, sums_arr):
    pav = np.asarray(pavT_arr).astype(np.float32).T  # [T, H]
    s = np.asarray(sums_arr).astype(np.float32).reshape(T, 1)
    return pav / s


def _in_map_for_core(inputs, b):
    return _make_in_maps(**inputs)[b]


def _out_from_core(sim):
    return _finalize(sim.tensor("pavT"), sim.tensor("sums"))


def kernel(x, Wq, Wk, Wv):
    global LAST_RESULT
    from concourse.bass_utils import run_bass_kernel_spmd

    in_maps = _make_in_maps(x, Wq, Wk, Wv)

    nc = _get_bass()
    res = run_bass_kernel_spmd(nc, in_maps, core_ids=list(range(N_CORES)))
    LAST_RESULT = res
    return np.stack(
        [_finalize(r["pavT"], r["sums"]) for r in res.results], axis=0
    )
